# revision 16
# baseline (speedup 1.0000x reference)
"""Trainium2 Bass kernel for DecoderWithAttention (Show-Attend-Tell decoder).

Sharding: data-parallel over batch B=128 -> 16 samples on each of 8 cores.
Per core, 20 recurrent attention+LSTM steps fully SBUF-resident (phase A),
then logits = h @ out_w.T with out_w streamed once from HBM (phase B).

Attention uses a "flat bn" layout: the (b, n) axis lives padded on
128-partition tiles [128, 25]; softmax runs there (exp without max shift --
scores are tiny, and softmax is shift-invariant so v_b is dropped too), and
the context contraction is 25 block-diagonal matmuls using a constant 0/1
indicator mask scaled by the attention weights.

kernel(**inputs) takes FULL unsharded inputs (as from setup_inputs()) and
returns the full [B, T, V] float32 logits.
"""

import os

os.environ.setdefault("MYCRO_LOCAL_CACHE", "1")

import numpy as np
import ml_dtypes

bf = ml_dtypes.bfloat16

# Problem dims (hardcoded per contract).
B, N, F, E, H, A, V, L = 128, 196, 512, 512, 512, 512, 30000, 21
T = L - 1            # 20 decode steps
NCORES = 8
BC = B // NCORES     # 16 samples per core
NF = BC * N          # 3136 flattened (b, n)
KT = 25              # ceil(NF / 128) flat bn tiles (last partial: 64 rows)
G = 4 * H            # 2048 gates
BT = BC * T          # 320 (t, b) rows per core

BROADCAST_ADD = True   # DVE tensor_tensor with free-dim step-0 broadcast


def build_program(has_bias: bool, has_outb: bool):
    import concourse.bacc as bacc
    import concourse.mybir as mybir
    import concourse.tile as tile

    dt = mybir.dt
    f32, f32r, b16 = dt.float32, dt.float32r, dt.bfloat16
    AF = mybir.ActivationFunctionType
    ALU = mybir.AluOpType
    AX = mybir.AxisListType

    nc = bacc.Bacc(
        "TRN2",
        target_bir_lowering=False,
        debug=False,
        enable_asserts=False,
        num_devices=NCORES,
    )

    def din(name, shape, dtype):
        return nc.dram_tensor(name, list(shape), dtype, kind="ExternalInput").ap()

    featsF_d = din("featsF", (128, KT, F), b16)       # flat bn tiles, padded
    featsT_d = din("featsT", (128, 4, NF), b16)       # [p, fk, bn] for fproj
    wft_d = din("wft", (128, 4, A), b16)
    wfb_d = din("wfb", (128, 4), f32)
    whb_d = din("whb", (128, 4), f32)
    whT_d = din("whT", (128, 4, A), b16)
    vt_d = din("vt", (128, 4), b16)
    eblk_d = din("eblk", (128, KT, BC), b16)          # indicator mask
    onesc_d = din("onesc", (128, 1), b16)
    wit_e_d = din("wit_e", (128, 4, G), b16)          # W_ih^T emb rows
    wit_c_d = din("wit_c", (128, 4, G), b16)         # W_ih^T ctx rows
    wht_d = din("wht", (128, 4, G), b16)
    emba_d = din("emba", (128, T, 4, BC), b16)
    h0t_d = din("h0t", (128, 4, BC), f32)
    c0_d = din("c0", (BC, H), f32)
    eyef_d = din("eyef", (BC, BC), f32)
    onesr_d = din("onesr", (1, 128), b16)
    biasrow_d = din("biasrow", (1, G), b16)
    owt_d = din("owt", (128, 4, V), b16)
    outb_d = din("outb", (1, V), b16)

    logits_d = nc.dram_tensor("logits", [BT, V], f32, kind="ExternalOutput").ap()

    with tile.TileContext(nc) as tc:
        with (
            tc.tile_pool(name="persist", bufs=1) as pp,
            tc.tile_pool(name="consts", bufs=1) as cp,
        ):
            hstate = pp.tile([128, 4, (T + 1) * BC], f32, tag="hstate")
            cst = pp.tile([BC, H], f32, tag="cst")
            srow = pp.tile([1, KT * 128], f32, tag="srow")

            eyef = cp.tile([BC, BC], f32, tag="eyef")
            vt = cp.tile([128, 4], b16, tag="vt")
            wfb = cp.tile([128, 4], f32, tag="wfb")
            whb = cp.tile([128, 4], f32, tag="whb")
            onesr = cp.tile([1, 128], b16, tag="onesr")
            onesc = cp.tile([128, 1], b16, tag="onesc")
            biasrow = cp.tile([1, G], b16, tag="biasrow")
            eblk = cp.tile([128, KT, BC], b16, tag="eblk")

            nc.sync.dma_start(eyef[:], eyef_d)
            nc.sync.dma_start(vt[:], vt_d)
            nc.sync.dma_start(wfb[:], wfb_d)
            nc.sync.dma_start(whb[:], whb_d)
            nc.sync.dma_start(onesr[:], onesr_d)
            nc.sync.dma_start(onesc[:], onesc_d)
            nc.sync.dma_start(biasrow[:], biasrow_d)
            nc.sync.dma_start(eblk[:], eblk_d)
            nc.sync.dma_start(hstate[:, :, 0:BC], h0t_d)
            nc.sync.dma_start(cst[:], c0_d)
            nc.vector.memset(srow[:], 0.0)

            with tc.tile_pool(name="phaseA", bufs=1) as pa:
                featsF = pa.tile([128, KT, F], b16, tag="featsF")
                fprojT = pa.tile([128, 4, BC, N], b16, tag="fprojT")
                wit_e = pa.tile([128, 4, G], b16, tag="wit_e")
                wit_c = pa.tile([128, 4, G], b16, tag="wit_c")
                wht_sb = pa.tile([128, 4, G], b16, tag="wht")
                whT_sb = pa.tile([128, 4, A], b16, tag="whT")
                emba = pa.tile([128, T, 4, BC], b16, tag="emba")

                nc.sync.dma_start(featsF[:], featsF_d)
                nc.sync.dma_start(wit_e[:], wit_e_d)
                nc.sync.dma_start(wit_c[:], wit_c_d)
                nc.sync.dma_start(wht_sb[:], wht_d)
                nc.sync.dma_start(whT_sb[:], whT_d)
                nc.sync.dma_start(emba[:], emba_d)

                # ---- setup: fprojT[a, bn] = Wf @ feats^T + Wf_b ----
                fprojT_fl = fprojT[:].rearrange("p a b n -> p a (b n)")
                with (
                    tc.tile_pool(name="setup", bufs=2) as sup,
                    tc.tile_pool(name="psF", bufs=2, space="PSUM") as psF,
                ):
                    wft_sb = sup.tile([128, 4, A], b16, tag="wft")
                    nc.sync.dma_start(wft_sb[:], wft_d)
                    CH = 512
                    nch = (NF + CH - 1) // CH
                    for j in range(nch):
                        c0c = j * CH
                        w = min(CH, NF - c0c)
                        ft = sup.tile([128, 4, CH], b16, tag="ft")
                        nc.sync.dma_start(
                            ft[:, :, 0:w], featsT_d[:, :, c0c : c0c + w]
                        )
                        for ac in range(4):
                            ps = psF.tile([128, CH], f32, tag="f")
                            for kk in range(4):
                                nc.tensor.matmul(
                                    ps[:, 0:w],
                                    wft_sb[:, kk, ac * 128 : (ac + 1) * 128],
                                    ft[:, kk, 0:w],
                                    start=(kk == 0),
                                    stop=(kk == 3),
                                )
                            nc.vector.tensor_scalar_add(
                                fprojT_fl[:, ac, c0c : c0c + w],
                                ps[:, 0:w],
                                wfb[:, ac : ac + 1],
                            )

                # ---- recurrent steps ----
                with (
                    tc.tile_pool(name="psM", bufs=1, space="PSUM") as psM,
                    tc.tile_pool(name="psS", bufs=3, space="PSUM") as psS,
                    tc.tile_pool(name="psC", bufs=1, space="PSUM") as psC,
                    tc.tile_pool(name="psT", bufs=1, space="PSUM") as psT,
                    tc.tile_pool(name="psG", bufs=2, space="PSUM") as psG,
                    tc.tile_pool(name="sm", bufs=2) as sm,
                    tc.tile_pool(name="smb", bufs=1) as smb,
                    tc.tile_pool(name="ep", bufs=2) as ep,
                    tc.tile_pool(name="et4", bufs=4) as et4,
                ):
                    for t in range(T):
                        hcol = hstate[:, :, t * BC : (t + 1) * BC]

                        # h in bf16 for the attention projection
                        h_b16 = sm.tile([128, 4, BC], b16, tag="h_b16")
                        nc.vector.tensor_copy(h_b16[:], hcol)
                        # hproj[a, b] = Wh @ h + Wh_b
                        hp = psM.tile([128, 4 * BC], f32, tag="m")
                        for ac in range(4):
                            for hk in range(4):
                                nc.tensor.matmul(
                                    hp[:, ac * BC : (ac + 1) * BC],
                                    whT_sb[:, hk, ac * 128 : (ac + 1) * 128],
                                    h_b16[:, hk, :],
                                    start=(hk == 0),
                                    stop=(hk == 3),
                                )
                        hp_sb = sm.tile([128, 4, BC], f32, tag="hp_sb")
                        for ac in range(4):
                            nc.vector.tensor_scalar_add(
                                hp_sb[:, ac, :],
                                hp[:, ac * BC : (ac + 1) * BC],
                                whb[:, ac : ac + 1],
                            )

                        # e = tanh(fproj + hproj)
                        e_ts = []
                        for ac in range(4):
                            e_in = ep.tile([128, BC, N], b16, tag="e_in")
                            if BROADCAST_ADD:
                                for hlf in range(2):
                                    hsl = slice(hlf * (BC // 2), (hlf + 1) * (BC // 2))
                                    h_bc = (
                                        hp_sb[:, ac, hsl]
                                        .unsqueeze(2)
                                        .broadcast_to((128, BC // 2, N))
                                    )
                                    nc.vector.tensor_add(
                                        e_in[:, hsl, :], fprojT[:, ac, hsl, :], h_bc
                                    )
                            else:
                                for b in range(BC):
                                    nc.vector.tensor_scalar_add(
                                        e_in[:, b, :],
                                        fprojT[:, ac, b, :],
                                        hp_sb[:, ac, b : b + 1],
                                    )
                            e_t = et4.tile([128, BC, N], b16, tag="e_t")
                            for hlf in range(2):
                                hsl = slice(hlf * (BC // 2), (hlf + 1) * (BC // 2))
                                nc.scalar.activation(
                                    e_t[:, hsl, :], e_in[:, hsl, :], AF.Tanh
                                )
                            e_ts.append(e_t)

                        # scores (flat bn) -> srow (staging row)
                        CH = 512
                        nch = (NF + CH - 1) // CH
                        for j in range(nch):
                            c0c = j * CH
                            w = min(CH, NF - c0c)
                            ps = psS.tile([1, CH], f32, tag="s")
                            for ac in range(4):
                                efl = e_ts[ac][:].rearrange(
                                    "p b n -> p (b n)"
                                )
                                nc.tensor.matmul(
                                    ps[0:1, 0:w],
                                    vt[:, ac : ac + 1],
                                    efl[:, c0c : c0c + w],
                                    start=(ac == 0),
                                    stop=(ac == 3),
                                )
                            nc.vector.tensor_copy(
                                srow[0:1, c0c : c0c + w], ps[0:1, 0:w]
                            )
                        # reshape srow -> [128, KT] via PE transposes
                        pst = psM.tile([128, KT], f32, tag="m")
                        for k in range(KT):
                            nc.tensor.transpose(
                                pst[:, k : k + 1],
                                srow[0:1, k * 128 : (k + 1) * 128],
                                eyef[0:1, 0:1],
                            )
                        # softmax pieces: exp (no max shift; scores are small)
                        expf = sm.tile([128, KT], f32, tag="expf")
                        nc.scalar.activation(expf[:], pst[:], AF.Exp)
                        # wblk[p, k, b] = eblk * expf (masked weights)
                        wblk = sm.tile([128, KT, BC], b16, tag="wblk")
                        if BROADCAST_ADD:
                            ex_bc = (
                                expf[:].unsqueeze(2).broadcast_to((128, KT, BC))
                            )
                            nc.vector.tensor_mul(wblk[:], eblk[:], ex_bc)
                        else:
                            for k in range(KT):
                                nc.vector.tensor_scalar_mul(
                                    wblk[:, k, :],
                                    eblk[:, k, :],
                                    expf[:, k : k + 1],
                                )

                        # sums + unnormalized ctx
                        psum_sum = psM.tile([BC, 1], f32, tag="m")
                        ps_cu = psC.tile([BC, F], f32, tag="c")
                        for k in range(KT):
                            nc.tensor.matmul(
                                psum_sum[:],
                                wblk[:, k, :],
                                onesc[:],
                                start=(k == 0),
                                stop=(k == KT - 1),
                            )
                            nc.tensor.matmul(
                                ps_cu[:],
                                wblk[:, k, :],
                                featsF[:, k, :],
                                start=(k == 0),
                                stop=(k == KT - 1),
                            )
                        rinv = sm.tile([BC, 1], f32, tag="rinv")
                        nc.vector.reciprocal(rinv[:], psum_sum[:])
                        ctx_sb = smb.tile([BC, F], f32, tag="ctx_sb")
                        nc.vector.tensor_scalar_mul(
                            ctx_sb[:], ps_cu[:], rinv[:]
                        )
                        xctx = sm.tile([128, 4, BC], b16, tag="xctx")
                        for kk in range(4):
                            pc = psT.tile([128, BC], f32, tag="tr")
                            nc.tensor.transpose(
                                pc[:], ctx_sb[:, kk * 128 : (kk + 1) * 128],
                                eyef[:],
                            )
                            nc.vector.tensor_copy(xctx[:, kk, :], pc[:])

                        # gates (i, f, g, o) + pointwise
                        tig = smb.tile([BC, 4, H], f32, tag="tig")
                        for g in range(4):
                            pg = psG.tile([BC, H], f32, tag="g")
                            gsl = slice(g * H, (g + 1) * H)
                            for kk in range(4):
                                nc.tensor.matmul(
                                    pg[:], emba[:, t, kk, :], wit_e[:, kk, gsl],
                                    start=(kk == 0), stop=False,
                                )
                            for kk in range(4):
                                nc.tensor.matmul(
                                    pg[:], xctx[:, kk, :], wit_c[:, kk, gsl],
                                    start=False, stop=False,
                                )
                            for hk in range(4):
                                last = (hk == 3) and not has_bias
                                nc.tensor.matmul(
                                    pg[:], h_b16[:, hk, :], wht_sb[:, hk, gsl],
                                    start=False, stop=last,
                                )
                            if has_bias:
                                nc.tensor.matmul(
                                    pg[:], onesr[0:1, 0:BC], biasrow[0:1, gsl],
                                    start=False, stop=True,
                                )
                            scale = 1.0 if g == 2 else 0.5
                            nc.scalar.activation(
                                tig[:, g, :], pg[:], AF.Tanh, scale=scale
                            )

                        # c = sig(f)*c + sig(i)*tanh(g); h = sig(o)*tanh(c)
                        for g in (0, 1, 3):
                            nc.vector.tensor_scalar(
                                tig[:, g, :], tig[:, g, :], 1.0, 0.5,
                                op0=ALU.add, op1=ALU.mult,
                            )
                        tmp1 = smb.tile([BC, H], f32, tag="u1")
                        nc.vector.tensor_mul(tmp1[:], tig[:, 1, :], cst[:])
                        tmp2 = smb.tile([BC, H], f32, tag="u2")
                        nc.vector.tensor_mul(tmp2[:], tig[:, 0, :], tig[:, 2, :])
                        nc.vector.tensor_add(cst[:], tmp1[:], tmp2[:])
                        tcn = smb.tile([BC, H], f32, tag="u1")
                        nc.scalar.activation(tcn[:], cst[:], AF.Tanh)
                        hn = smb.tile([BC, H], f32, tag="u2")
                        nc.vector.tensor_mul(hn[:], tig[:, 3, :], tcn[:])

                        for kk in range(4):
                            ph = psT.tile([128, BC], f32, tag="tr")
                            nc.tensor.transpose(
                                ph[:], hn[:, kk * 128 : (kk + 1) * 128], eyef[:]
                            )
                            nc.vector.tensor_copy(
                                hstate[:, kk, (t + 1) * BC : (t + 2) * BC],
                                ph[:],
                            )

            # ---- phase B: logits = h @ out_w^T (+ out_b) ----
            with (
                tc.tile_pool(name="owp", bufs=3) as owp,
                tc.tile_pool(name="psB", bufs=4, space="PSUM") as psB,
                tc.tile_pool(name="obp", bufs=1) as obp,
                tc.tile_pool(name="lbp", bufs=4) as lbp,
            ):
                hb16 = obp.tile([128, 4, T * BC], b16, tag="hb16")
                nc.vector.tensor_copy(hb16[:], hstate[:, :, BC:])
                if has_outb:
                    outb_sb = obp.tile([1, V], b16, tag="outb")
                    nc.sync.dma_start(outb_sb[:], outb_d)
                VC = 2048
                nvc = (V + VC - 1) // VC
                mtiles = [(0, 128), (128, 128), (256, 64)]
                for vc in range(nvc):
                    v0 = vc * VC
                    vw = min(VC, V - v0)
                    ow = owp.tile([128, 4, VC], b16, tag="ow")
                    nc.sync.dma_start(ow[:, :, 0:vw], owt_d[:, :, v0 : v0 + vw])
                    for m0, mr in mtiles:
                        lb = lbp.tile([128, VC], f32, tag="lb")
                        for half in range(0, vw, 512):
                            hw_ = min(512, vw - half)
                            ps = psB.tile([128, 512], f32, tag="pb")
                            for hk in range(4):
                                last = (hk == 3) and not has_outb
                                nc.tensor.matmul(
                                    ps[0:mr, 0:hw_],
                                    hb16[:, hk, m0 : m0 + mr],
                                    ow[:, hk, half : half + hw_],
                                    start=(hk == 0), stop=last,
                                )
                            if has_outb:
                                nc.tensor.matmul(
                                    ps[0:mr, 0:hw_],
                                    onesr[0:1, 0:mr],
                                    outb_sb[0:1, v0 + half : v0 + half + hw_],
                                    start=False, stop=True,
                                )
                            nc.vector.tensor_copy(
                                lb[0:mr, half : half + hw_], ps[0:mr, 0:hw_]
                            )
                        nc.sync.dma_start(
                            logits_d[m0 : m0 + mr, v0 : v0 + vw],
                            lb[0:mr, 0:vw],
                        )

    nc.compile()
    return nc


def _tileize(mat, ntiles):
    """[K, M] -> [128, ntiles, M]: row k = kk*128+p -> [p, kk, :]."""
    K, M = mat.shape
    assert K == ntiles * 128
    return np.ascontiguousarray(mat.reshape(ntiles, 128, M).transpose(1, 0, 2))


def prep_inputs(feats, captions, embed_W, Wf_w, Wf_b, Wh_w, Wh_b, v_w, v_b,
                W_ih, W_hh, b_ih, b_hh, init_w, init_b, initc_w, initc_b,
                out_w, out_b):
    feats = np.asarray(feats, np.float32)
    captions = np.asarray(captions)
    emtab = np.array(embed_W, np.float32)
    emtab[0] = 0.0
    emb = emtab[captions[:, :-1]]                      # [B, T, E]
    gfeat = feats.mean(axis=1, dtype=np.float32)
    h0 = (gfeat @ np.asarray(init_w, np.float32).T + init_b).astype(np.float32)
    c0 = (gfeat @ np.asarray(initc_w, np.float32).T + initc_b).astype(np.float32)

    witT = np.asarray(W_ih, np.float32).T              # [E+F, G]
    wit_e = _tileize(witT[0:E], 4).astype(bf)
    wit_c = _tileize(witT[E:], 4).astype(bf)
    wht = _tileize(np.asarray(W_hh, np.float32).T, 4).astype(bf)
    wft = _tileize(np.asarray(Wf_w, np.float32).T, 4).astype(bf)
    whT = _tileize(np.asarray(Wh_w, np.float32).T, 4).astype(bf)
    owt = _tileize(np.asarray(out_w, np.float32).T, 4).astype(bf)
    wfb = np.ascontiguousarray(np.asarray(Wf_b, np.float32).reshape(4, 128).T)
    whb = np.ascontiguousarray(np.asarray(Wh_b, np.float32).reshape(4, 128).T)
    vt = np.ascontiguousarray(
        np.asarray(v_w, np.float32)[0].reshape(4, 128).T
    ).astype(bf)
    biasrow = (np.asarray(b_ih, np.float32) + np.asarray(b_hh, np.float32))[
        None, :
    ].astype(bf)
    outb = np.asarray(out_b, np.float32)[None, :].astype(bf)
    onesr = np.ones((1, 128), bf)
    onesc = np.ones((128, 1), bf)
    eyef = np.eye(BC, dtype=np.float32)

    # indicator: flat row 128k+p belongs to sample b
    idx = np.arange(KT * 128)
    bown = idx // N
    eblk = np.zeros((KT * 128, BC), np.float32)
    valid = idx < NF
    eblk[valid, bown[valid]] = 1.0
    eblk = np.ascontiguousarray(
        eblk.reshape(KT, 128, BC).transpose(1, 0, 2)
    ).astype(bf)

    # NOTE: v_b shifts all scores of a row equally -> softmax-invariant.

    in_maps = []
    for c in range(NCORES):
        s = slice(c * BC, (c + 1) * BC)
        fc = feats[s]                                   # [16, 196, 512]
        ff = fc.reshape(NF, F)
        featsF = np.zeros((KT * 128, F), np.float32)
        featsF[0:NF] = ff
        featsF = np.ascontiguousarray(
            featsF.reshape(KT, 128, F).transpose(1, 0, 2)
        ).astype(bf)
        featsT = np.ascontiguousarray(
            ff.T.reshape(4, 128, NF).transpose(1, 0, 2)
        ).astype(bf)
        emba = np.ascontiguousarray(
            emb[s].reshape(BC, T, 4, 128).transpose(3, 1, 2, 0)
        ).astype(bf)
        h0t = np.ascontiguousarray(
            h0[s].reshape(BC, 4, 128).transpose(2, 1, 0)
        )
        in_maps.append({
            "featsF": featsF, "featsT": featsT, "wft": wft, "wfb": wfb,
            "whb": whb, "whT": whT, "vt": vt, "eblk": eblk, "onesc": onesc,
            "wit_e": wit_e, "wit_c": wit_c, "wht": wht, "emba": emba,
            "h0t": h0t, "c0": np.ascontiguousarray(c0[s]), "eyef": eyef,
            "onesr": onesr, "biasrow": biasrow, "owt": owt, "outb": outb,
        })
    return in_maps, bool(np.any(biasrow)), bool(np.any(outb))


def assemble(results):
    out = np.empty((B, T, V), np.float32)
    for c, r in enumerate(results):
        out[c * BC : (c + 1) * BC] = (
            r["logits"].reshape(T, BC, V).transpose(1, 0, 2)
        )
    return out


_CACHE = {}
LAST_RESULTS = None


def kernel(**inputs):
    global LAST_RESULTS
    from concourse.bass_utils import run_bass_kernel_spmd

    in_maps, has_bias, has_outb = prep_inputs(**inputs)
    key = (has_bias, has_outb)
    if key not in _CACHE:
        _CACHE[key] = build_program(has_bias, has_outb)
    res = run_bass_kernel_spmd(
        _CACHE[key], in_maps, list(range(NCORES)),
        trace=bool(int(os.environ.get("KTRACE", "0"))),
    )
    LAST_RESULTS = res
    return assemble(res.results)


# revision 20
# speedup vs baseline: 1.0175x; 1.0175x over previous
"""Trainium2 Bass kernel for DecoderWithAttention (Show-Attend-Tell decoder).

Sharding: data-parallel over batch B=128 -> 16 samples on each of 8 cores.
Per core, 20 recurrent attention+LSTM steps fully SBUF-resident (phase A),
then logits = h @ out_w.T with out_w streamed once from HBM (phase B).

Attention uses a "flat bn" layout: the (b, n) axis lives padded on
128-partition tiles [128, 25]; softmax runs there (exp without max shift --
scores are tiny, and softmax is shift-invariant so v_b is dropped too), and
the context contraction is 25 block-diagonal matmuls using a constant 0/1
indicator mask scaled by the attention weights.

kernel(**inputs) takes FULL unsharded inputs (as from setup_inputs()) and
returns the full [B, T, V] float32 logits.
"""

import os

os.environ.setdefault("MYCRO_LOCAL_CACHE", "1")

import numpy as np
import ml_dtypes

bf = ml_dtypes.bfloat16

# Problem dims (hardcoded per contract).
B, N, F, E, H, A, V, L = 128, 196, 512, 512, 512, 512, 30000, 21
T = L - 1            # 20 decode steps
NCORES = 8
BC = B // NCORES     # 16 samples per core
NF = BC * N          # 3136 flattened (b, n)
KT = 25              # ceil(NF / 128) flat bn tiles (last partial: 64 rows)
G = 4 * H            # 2048 gates
BT = BC * T          # 320 (t, b) rows per core

BROADCAST_ADD = True   # DVE tensor_tensor with free-dim step-0 broadcast


def build_program(has_bias: bool, has_outb: bool):
    import concourse.bacc as bacc
    import concourse.mybir as mybir
    import concourse.tile as tile

    dt = mybir.dt
    f32, f32r, b16 = dt.float32, dt.float32r, dt.bfloat16
    AF = mybir.ActivationFunctionType
    ALU = mybir.AluOpType
    AX = mybir.AxisListType

    nc = bacc.Bacc(
        "TRN2",
        target_bir_lowering=False,
        debug=False,
        enable_asserts=False,
        num_devices=NCORES,
    )

    def din(name, shape, dtype):
        return nc.dram_tensor(name, list(shape), dtype, kind="ExternalInput").ap()

    featsF_d = din("featsF", (128, KT, F), b16)       # flat bn tiles, padded
    featsT_d = din("featsT", (128, 4, NF), b16)       # [p, fk, bn] for fproj
    wft_d = din("wft", (128, 4, A), b16)
    wfb_d = din("wfb", (128, 4), f32)
    whb_d = din("whb", (128, 4), f32)
    whT_d = din("whT", (128, 4, A), b16)
    vt_d = din("vt", (128, 4), b16)
    eblk_d = din("eblk", (128, KT, BC), b16)          # indicator mask
    onesc_d = din("onesc", (128, 1), b16)
    wit_e_d = din("wit_e", (128, 4, G), b16)          # W_ih^T emb rows
    wit_c_d = din("wit_c", (128, 4, G), b16)         # W_ih^T ctx rows
    wht_d = din("wht", (128, 4, G), b16)
    emba_d = din("emba", (128, T, 4, BC), b16)
    h0t_d = din("h0t", (128, 4, BC), f32)
    c0_d = din("c0", (BC, H), f32)
    eyef_d = din("eyef", (BC, BC), f32)
    onesr_d = din("onesr", (1, 128), b16)
    biasrow_d = din("biasrow", (1, G), b16)
    owt_d = din("owt", (128, 4, V), b16)
    outb_d = din("outb", (1, V), b16)

    logits_d = nc.dram_tensor("logits", [BT, V], f32, kind="ExternalOutput").ap()

    with tile.TileContext(nc) as tc:
        with (
            tc.tile_pool(name="persist", bufs=1) as pp,
            tc.tile_pool(name="consts", bufs=1) as cp,
        ):
            hstate = pp.tile([128, 4, (T + 1) * BC], f32, tag="hstate")
            cst = pp.tile([BC, H], f32, tag="cst")
            srow = pp.tile([1, KT * 128], f32, tag="srow")

            eyef = cp.tile([BC, BC], f32, tag="eyef")
            vt = cp.tile([128, 4], b16, tag="vt")
            wfb = cp.tile([128, 4], f32, tag="wfb")
            whb = cp.tile([128, 4], f32, tag="whb")
            onesr = cp.tile([1, 128], b16, tag="onesr")
            onesc = cp.tile([128, 1], b16, tag="onesc")
            biasrow = cp.tile([1, G], b16, tag="biasrow")
            eblk = cp.tile([128, KT, BC], b16, tag="eblk")

            nc.sync.dma_start(eyef[:], eyef_d)
            nc.sync.dma_start(vt[:], vt_d)
            nc.sync.dma_start(wfb[:], wfb_d)
            nc.sync.dma_start(whb[:], whb_d)
            nc.sync.dma_start(onesr[:], onesr_d)
            nc.sync.dma_start(onesc[:], onesc_d)
            nc.sync.dma_start(biasrow[:], biasrow_d)
            nc.sync.dma_start(eblk[:], eblk_d)
            nc.sync.dma_start(hstate[:, :, 0:BC], h0t_d)
            nc.sync.dma_start(cst[:], c0_d)
            nc.vector.memset(srow[:], 0.0)

            with tc.tile_pool(name="phaseA", bufs=1) as pa:
                featsF = pa.tile([128, KT, F], b16, tag="featsF")
                fprojT = pa.tile([128, 4, BC, N], b16, tag="fprojT")
                wit_e = pa.tile([128, 4, G], b16, tag="wit_e")
                wit_c = pa.tile([128, 4, G], b16, tag="wit_c")
                wht_sb = pa.tile([128, 4, G], b16, tag="wht")
                whT_sb = pa.tile([128, 4, A], b16, tag="whT")
                emba = pa.tile([128, T, 4, BC], b16, tag="emba")

                nc.sync.dma_start(featsF[:], featsF_d)
                nc.sync.dma_start(wit_e[:], wit_e_d)
                nc.sync.dma_start(wit_c[:], wit_c_d)
                nc.sync.dma_start(wht_sb[:], wht_d)
                nc.sync.dma_start(whT_sb[:], whT_d)
                nc.sync.dma_start(emba[:], emba_d)

                # ---- setup: fprojT[a, bn] = Wf @ feats^T + Wf_b ----
                fprojT_fl = fprojT[:].rearrange("p a b n -> p a (b n)")
                with (
                    tc.tile_pool(name="setup", bufs=2) as sup,
                    tc.tile_pool(name="psF", bufs=2, space="PSUM") as psF,
                ):
                    wft_sb = sup.tile([128, 4, A], b16, tag="wft")
                    nc.sync.dma_start(wft_sb[:], wft_d)
                    CH = 512
                    nch = (NF + CH - 1) // CH
                    for j in range(nch):
                        c0c = j * CH
                        w = min(CH, NF - c0c)
                        ft = sup.tile([128, 4, CH], b16, tag="ft")
                        nc.sync.dma_start(
                            ft[:, :, 0:w], featsT_d[:, :, c0c : c0c + w]
                        )
                        for ac in range(4):
                            ps = psF.tile([128, CH], f32, tag="f")
                            for kk in range(4):
                                nc.tensor.matmul(
                                    ps[:, 0:w],
                                    wft_sb[:, kk, ac * 128 : (ac + 1) * 128],
                                    ft[:, kk, 0:w],
                                    start=(kk == 0),
                                    stop=(kk == 3),
                                )
                            nc.vector.tensor_scalar_add(
                                fprojT_fl[:, ac, c0c : c0c + w],
                                ps[:, 0:w],
                                wfb[:, ac : ac + 1],
                            )

                # ---- recurrent steps ----
                with (
                    tc.tile_pool(name="psM", bufs=1, space="PSUM") as psM,
                    tc.tile_pool(name="psS", bufs=2, space="PSUM") as psS,
                    tc.tile_pool(name="psC", bufs=1, space="PSUM") as psC,
                    tc.tile_pool(name="psT", bufs=1, space="PSUM") as psT,
                    tc.tile_pool(name="psG", bufs=2, space="PSUM") as psG,
                    tc.tile_pool(name="psB2", bufs=1, space="PSUM") as psB2,
                    tc.tile_pool(name="sm", bufs=2) as sm,
                    tc.tile_pool(name="smb", bufs=1) as smb,
                    tc.tile_pool(name="ep", bufs=2) as ep,
                    tc.tile_pool(name="et4", bufs=4) as et4,
                    tc.tile_pool(name="ilv", bufs=2) as ilv,
                    tc.tile_pool(name="ilh", bufs=1) as ilh,
                ):
                    # interleaved phase-B for logits rows 0..127 (t=0..7):
                    # spread over the PE gaps of steps 8..19
                    VCI = 512
                    nvci = (V + VCI - 1) // VCI
                    ilv_plan = {}
                    done = 0
                    for t in range(8, T):
                        n = min(nvci - done, (nvci + (T - 8) - 1) // (T - 8))
                        ilv_plan[t] = range(done, done + n)
                        done += n
                    hm0 = None
                    for t in range(T):
                        hcol = hstate[:, :, t * BC : (t + 1) * BC]

                        # h in bf16 for the attention projection
                        h_b16 = sm.tile([128, 4, BC], b16, tag="h_b16")
                        nc.vector.tensor_copy(h_b16[:], hcol)
                        # hproj[a, b] = Wh @ h + Wh_b
                        hp = psM.tile([128, 4 * BC], f32, tag="m")
                        for ac in range(4):
                            for hk in range(4):
                                nc.tensor.matmul(
                                    hp[:, ac * BC : (ac + 1) * BC],
                                    whT_sb[:, hk, ac * 128 : (ac + 1) * 128],
                                    h_b16[:, hk, :],
                                    start=(hk == 0),
                                    stop=(hk == 3),
                                )
                        hp_sb = sm.tile([128, 4, BC], f32, tag="hp_sb")
                        for ac in range(4):
                            nc.vector.tensor_scalar_add(
                                hp_sb[:, ac, :],
                                hp[:, ac * BC : (ac + 1) * BC],
                                whb[:, ac : ac + 1],
                            )

                        # e = tanh(fproj + hproj)
                        e_ts = []
                        for ac in range(4):
                            e_in = ep.tile([128, BC, N], b16, tag="e_in")
                            if BROADCAST_ADD:
                                for hlf in range(2):
                                    hsl = slice(hlf * (BC // 2), (hlf + 1) * (BC // 2))
                                    h_bc = (
                                        hp_sb[:, ac, hsl]
                                        .unsqueeze(2)
                                        .broadcast_to((128, BC // 2, N))
                                    )
                                    nc.vector.tensor_add(
                                        e_in[:, hsl, :], fprojT[:, ac, hsl, :], h_bc
                                    )
                            else:
                                for b in range(BC):
                                    nc.vector.tensor_scalar_add(
                                        e_in[:, b, :],
                                        fprojT[:, ac, b, :],
                                        hp_sb[:, ac, b : b + 1],
                                    )
                            e_t = et4.tile([128, BC, N], b16, tag="e_t")
                            for hlf in range(2):
                                hsl = slice(hlf * (BC // 2), (hlf + 1) * (BC // 2))
                                nc.scalar.activation(
                                    e_t[:, hsl, :], e_in[:, hsl, :], AF.Tanh
                                )
                            e_ts.append(e_t)

                        # scores (flat bn) -> srow (staging row)
                        CH = 512
                        nch = (NF + CH - 1) // CH
                        for j in range(nch):
                            c0c = j * CH
                            w = min(CH, NF - c0c)
                            ps = psS.tile([1, CH], f32, tag="s")
                            for ac in range(4):
                                efl = e_ts[ac][:].rearrange(
                                    "p b n -> p (b n)"
                                )
                                nc.tensor.matmul(
                                    ps[0:1, 0:w],
                                    vt[:, ac : ac + 1],
                                    efl[:, c0c : c0c + w],
                                    start=(ac == 0),
                                    stop=(ac == 3),
                                )
                            nc.vector.tensor_copy(
                                srow[0:1, c0c : c0c + w], ps[0:1, 0:w]
                            )
                        # reshape srow -> [128, KT] via PE transposes
                        pst = psM.tile([128, KT], f32, tag="m")
                        for k in range(KT):
                            nc.tensor.transpose(
                                pst[:, k : k + 1],
                                srow[0:1, k * 128 : (k + 1) * 128],
                                eyef[0:1, 0:1],
                            )
                        # softmax pieces: exp (no max shift; scores are small)
                        expf = sm.tile([128, KT], f32, tag="expf")
                        nc.scalar.activation(expf[:], pst[:], AF.Exp)
                        # wblk[p, k, b] = eblk * expf (masked weights)
                        wblk = sm.tile([128, KT, BC], b16, tag="wblk")
                        if BROADCAST_ADD:
                            ex_bc = (
                                expf[:].unsqueeze(2).broadcast_to((128, KT, BC))
                            )
                            nc.vector.tensor_mul(wblk[:], eblk[:], ex_bc)
                        else:
                            for k in range(KT):
                                nc.vector.tensor_scalar_mul(
                                    wblk[:, k, :],
                                    eblk[:, k, :],
                                    expf[:, k : k + 1],
                                )

                        # normalizer: one matmul -> per-(k,b) column sums,
                        # segmented reduce over k, transpose back to [b, 1]
                        psum_sum = psM.tile([1, KT * BC], f32, tag="m")
                        nc.tensor.matmul(
                            psum_sum[:],
                            onesc[:],
                            wblk[:].rearrange("p k b -> p (k b)"),
                            start=True,
                            stop=True,
                        )
                        sumrow = sm.tile([1, BC], f32, tag="sumrow")
                        nc.vector.tensor_reduce(
                            sumrow[:].unsqueeze(2),
                            psum_sum[:].rearrange("p (k b) -> p b k", k=KT),
                            axis=AX.X,
                            op=ALU.add,
                        )
                        psr = psM.tile([BC, 1], f32, tag="m")
                        nc.tensor.transpose(
                            psr[:], sumrow[:], eyef[0:1, 0:1]
                        )
                        rinv = sm.tile([BC, 1], f32, tag="rinv")
                        nc.vector.reciprocal(rinv[:], psr[:])
                        # unnormalized ctx
                        ps_cu = psC.tile([BC, F], f32, tag="c")
                        for k in range(KT):
                            nc.tensor.matmul(
                                ps_cu[:],
                                wblk[:, k, :],
                                featsF[:, k, :],
                                start=(k == 0),
                                stop=(k == KT - 1),
                            )
                        ctx_sb = smb.tile([BC, F], f32, tag="ctx_sb")
                        nc.vector.tensor_scalar_mul(
                            ctx_sb[:], ps_cu[:], rinv[:]
                        )
                        xctx = sm.tile([128, 4, BC], b16, tag="xctx")
                        for kk in range(4):
                            pc = psT.tile([128, BC], f32, tag="tr")
                            nc.tensor.transpose(
                                pc[:], ctx_sb[:, kk * 128 : (kk + 1) * 128],
                                eyef[:],
                            )
                            nc.vector.tensor_copy(xctx[:, kk, :], pc[:])

                        # gates (i, f, g, o) + pointwise
                        tig = smb.tile([BC, 4, H], f32, tag="tig")
                        for g in range(4):
                            pg = psG.tile([BC, H], f32, tag="g")
                            gsl = slice(g * H, (g + 1) * H)
                            for kk in range(4):
                                nc.tensor.matmul(
                                    pg[:], emba[:, t, kk, :], wit_e[:, kk, gsl],
                                    start=(kk == 0), stop=False,
                                )
                            for kk in range(4):
                                nc.tensor.matmul(
                                    pg[:], xctx[:, kk, :], wit_c[:, kk, gsl],
                                    start=False, stop=False,
                                )
                            for hk in range(4):
                                last = (hk == 3) and not has_bias
                                nc.tensor.matmul(
                                    pg[:], h_b16[:, hk, :], wht_sb[:, hk, gsl],
                                    start=False, stop=last,
                                )
                            if has_bias:
                                nc.tensor.matmul(
                                    pg[:], onesr[0:1, 0:BC], biasrow[0:1, gsl],
                                    start=False, stop=True,
                                )
                            scale = 1.0 if g == 2 else 0.5
                            nc.scalar.activation(
                                tig[:, g, :], pg[:], AF.Tanh, scale=scale
                            )

                        # c = sig(f)*c + sig(i)*tanh(g); h = sig(o)*tanh(c)
                        for g in (0, 1, 3):
                            nc.vector.tensor_scalar(
                                tig[:, g, :], tig[:, g, :], 1.0, 0.5,
                                op0=ALU.add, op1=ALU.mult,
                            )
                        tmp1 = smb.tile([BC, H], f32, tag="u1")
                        nc.vector.tensor_mul(tmp1[:], tig[:, 1, :], cst[:])
                        tmp2 = smb.tile([BC, H], f32, tag="u2")
                        nc.vector.tensor_mul(tmp2[:], tig[:, 0, :], tig[:, 2, :])
                        nc.vector.tensor_add(cst[:], tmp1[:], tmp2[:])
                        tcn = smb.tile([BC, H], f32, tag="u1")
                        nc.scalar.activation(tcn[:], cst[:], AF.Tanh)
                        hn = smb.tile([BC, H], f32, tag="u2")
                        nc.vector.tensor_mul(hn[:], tig[:, 3, :], tcn[:])

                        for kk in range(4):
                            ph = psT.tile([128, BC], f32, tag="tr")
                            nc.tensor.transpose(
                                ph[:], hn[:, kk * 128 : (kk + 1) * 128], eyef[:]
                            )
                            nc.vector.tensor_copy(
                                hstate[:, kk, (t + 1) * BC : (t + 2) * BC],
                                ph[:],
                            )
                        if t == 7:
                            hm0 = ilh.tile([128, 4, 128], b16, tag="hm0")
                            nc.vector.tensor_copy(
                                hm0[:], hstate[:, :, BC : BC + 128]
                            )
                        for ic, vc in enumerate(ilv_plan.get(t, ())):
                            v0 = vc * VCI
                            vw = min(VCI, V - v0)
                            ow2 = ilv.tile([128, 4, VCI], b16, tag="ow2")
                            nc.sync.dma_start(
                                ow2[:, :, 0:vw], owt_d[:, :, v0 : v0 + vw]
                            )
                            pb = psB2.tile([128, VCI], f32, tag="b2")
                            for hk in range(4):
                                nc.tensor.matmul(
                                    pb[:, 0:vw], hm0[:, hk, :],
                                    ow2[:, hk, 0:vw],
                                    start=(hk == 0), stop=(hk == 3),
                                )
                            lb2 = ilv.tile([128, VCI], f32, tag="lb2")
                            if ic % 2 == 0:
                                nc.vector.tensor_copy(lb2[:, 0:vw], pb[:, 0:vw])
                            else:
                                nc.scalar.copy(lb2[:, 0:vw], pb[:, 0:vw])
                            nc.sync.dma_start(
                                logits_d[0:128, v0 : v0 + vw], lb2[:, 0:vw]
                            )

            # ---- phase B: logits = h @ out_w^T (+ out_b) ----
            with (
                tc.tile_pool(name="owp", bufs=3) as owp,
                tc.tile_pool(name="psB", bufs=4, space="PSUM") as psB,
                tc.tile_pool(name="obp", bufs=1) as obp,
                tc.tile_pool(name="lbp", bufs=4) as lbp,
            ):
                hb16 = obp.tile([128, 4, T * BC], b16, tag="hb16")
                nc.vector.tensor_copy(hb16[:], hstate[:, :, BC:])
                if has_outb:
                    outb_sb = obp.tile([1, V], b16, tag="outb")
                    nc.sync.dma_start(outb_sb[:], outb_d)
                VC = 2048
                nvc = (V + VC - 1) // VC
                mtiles = [(128, 128), (256, 64)]
                for vc in range(nvc):
                    v0 = vc * VC
                    vw = min(VC, V - v0)
                    ow = owp.tile([128, 4, VC], b16, tag="ow")
                    nc.sync.dma_start(ow[:, :, 0:vw], owt_d[:, :, v0 : v0 + vw])
                    for m0, mr in mtiles:
                        lb = lbp.tile([128, VC], f32, tag="lb")
                        for half in range(0, vw, 512):
                            hw_ = min(512, vw - half)
                            ps = psB.tile([128, 512], f32, tag="pb")
                            for hk in range(4):
                                last = (hk == 3) and not has_outb
                                nc.tensor.matmul(
                                    ps[0:mr, 0:hw_],
                                    hb16[:, hk, m0 : m0 + mr],
                                    ow[:, hk, half : half + hw_],
                                    start=(hk == 0), stop=last,
                                )
                            if has_outb:
                                nc.tensor.matmul(
                                    ps[0:mr, 0:hw_],
                                    onesr[0:1, 0:mr],
                                    outb_sb[0:1, v0 + half : v0 + half + hw_],
                                    start=False, stop=True,
                                )
                            nc.vector.tensor_copy(
                                lb[0:mr, half : half + hw_], ps[0:mr, 0:hw_]
                            )
                        nc.sync.dma_start(
                            logits_d[m0 : m0 + mr, v0 : v0 + vw],
                            lb[0:mr, 0:vw],
                        )

    nc.compile()
    return nc


def _tileize(mat, ntiles):
    """[K, M] -> [128, ntiles, M]: row k = kk*128+p -> [p, kk, :]."""
    K, M = mat.shape
    assert K == ntiles * 128
    return np.ascontiguousarray(mat.reshape(ntiles, 128, M).transpose(1, 0, 2))


def prep_inputs(feats, captions, embed_W, Wf_w, Wf_b, Wh_w, Wh_b, v_w, v_b,
                W_ih, W_hh, b_ih, b_hh, init_w, init_b, initc_w, initc_b,
                out_w, out_b):
    feats = np.asarray(feats, np.float32)
    captions = np.asarray(captions)
    emtab = np.array(embed_W, np.float32)
    emtab[0] = 0.0
    emb = emtab[captions[:, :-1]]                      # [B, T, E]
    gfeat = feats.mean(axis=1, dtype=np.float32)
    h0 = (gfeat @ np.asarray(init_w, np.float32).T + init_b).astype(np.float32)
    c0 = (gfeat @ np.asarray(initc_w, np.float32).T + initc_b).astype(np.float32)

    witT = np.asarray(W_ih, np.float32).T              # [E+F, G]
    wit_e = _tileize(witT[0:E], 4).astype(bf)
    wit_c = _tileize(witT[E:], 4).astype(bf)
    wht = _tileize(np.asarray(W_hh, np.float32).T, 4).astype(bf)
    wft = _tileize(np.asarray(Wf_w, np.float32).T, 4).astype(bf)
    whT = _tileize(np.asarray(Wh_w, np.float32).T, 4).astype(bf)
    owt = _tileize(np.asarray(out_w, np.float32).T, 4).astype(bf)
    wfb = np.ascontiguousarray(np.asarray(Wf_b, np.float32).reshape(4, 128).T)
    whb = np.ascontiguousarray(np.asarray(Wh_b, np.float32).reshape(4, 128).T)
    vt = np.ascontiguousarray(
        np.asarray(v_w, np.float32)[0].reshape(4, 128).T
    ).astype(bf)
    biasrow = (np.asarray(b_ih, np.float32) + np.asarray(b_hh, np.float32))[
        None, :
    ].astype(bf)
    outb = np.asarray(out_b, np.float32)[None, :].astype(bf)
    onesr = np.ones((1, 128), bf)
    onesc = np.ones((128, 1), bf)
    eyef = np.eye(BC, dtype=np.float32)

    # indicator: flat row 128k+p belongs to sample b
    idx = np.arange(KT * 128)
    bown = idx // N
    eblk = np.zeros((KT * 128, BC), np.float32)
    valid = idx < NF
    eblk[valid, bown[valid]] = 1.0
    eblk = np.ascontiguousarray(
        eblk.reshape(KT, 128, BC).transpose(1, 0, 2)
    ).astype(bf)

    # NOTE: v_b shifts all scores of a row equally -> softmax-invariant.

    in_maps = []
    for c in range(NCORES):
        s = slice(c * BC, (c + 1) * BC)
        fc = feats[s]                                   # [16, 196, 512]
        ff = fc.reshape(NF, F)
        featsF = np.zeros((KT * 128, F), np.float32)
        featsF[0:NF] = ff
        featsF = np.ascontiguousarray(
            featsF.reshape(KT, 128, F).transpose(1, 0, 2)
        ).astype(bf)
        featsT = np.ascontiguousarray(
            ff.T.reshape(4, 128, NF).transpose(1, 0, 2)
        ).astype(bf)
        emba = np.ascontiguousarray(
            emb[s].reshape(BC, T, 4, 128).transpose(3, 1, 2, 0)
        ).astype(bf)
        h0t = np.ascontiguousarray(
            h0[s].reshape(BC, 4, 128).transpose(2, 1, 0)
        )
        in_maps.append({
            "featsF": featsF, "featsT": featsT, "wft": wft, "wfb": wfb,
            "whb": whb, "whT": whT, "vt": vt, "eblk": eblk, "onesc": onesc,
            "wit_e": wit_e, "wit_c": wit_c, "wht": wht, "emba": emba,
            "h0t": h0t, "c0": np.ascontiguousarray(c0[s]), "eyef": eyef,
            "onesr": onesr, "biasrow": biasrow, "owt": owt, "outb": outb,
        })
    return in_maps, bool(np.any(biasrow)), bool(np.any(outb))


def assemble(results):
    out = np.empty((B, T, V), np.float32)
    for c, r in enumerate(results):
        out[c * BC : (c + 1) * BC] = (
            r["logits"].reshape(T, BC, V).transpose(1, 0, 2)
        )
    return out


_CACHE = {}
LAST_RESULTS = None


def kernel(**inputs):
    global LAST_RESULTS
    from concourse.bass_utils import run_bass_kernel_spmd

    in_maps, has_bias, has_outb = prep_inputs(**inputs)
    key = (has_bias, has_outb)
    if key not in _CACHE:
        _CACHE[key] = build_program(has_bias, has_outb)
    res = run_bass_kernel_spmd(
        _CACHE[key], in_maps, list(range(NCORES)),
        trace=bool(int(os.environ.get("KTRACE", "0"))),
    )
    LAST_RESULTS = res
    return assemble(res.results)


# revision 22
# speedup vs baseline: 1.0175x; 1.0000x over previous
"""Trainium2 Bass kernel for DecoderWithAttention (Show-Attend-Tell decoder).

Sharding: data-parallel over batch B=128 -> 16 samples on each of 8 cores.
Per core, 20 recurrent attention+LSTM steps fully SBUF-resident (phase A),
then logits = h @ out_w.T with out_w streamed once from HBM (phase B).

Attention uses a "flat bn" layout: the (b, n) axis lives padded on
128-partition tiles [128, 25]; softmax runs there (exp without max shift --
scores are tiny, and softmax is shift-invariant so v_b is dropped too), and
the context contraction is 25 block-diagonal matmuls using a constant 0/1
indicator mask scaled by the attention weights.

kernel(**inputs) takes FULL unsharded inputs (as from setup_inputs()) and
returns the full [B, T, V] float32 logits.
"""

import os

os.environ.setdefault("MYCRO_LOCAL_CACHE", "1")

import numpy as np
import ml_dtypes

bf = ml_dtypes.bfloat16

# Problem dims (hardcoded per contract).
B, N, F, E, H, A, V, L = 128, 196, 512, 512, 512, 512, 30000, 21
T = L - 1            # 20 decode steps
NCORES = 8
BC = B // NCORES     # 16 samples per core
NF = BC * N          # 3136 flattened (b, n)
KT = 25              # ceil(NF / 128) flat bn tiles (last partial: 64 rows)
G = 4 * H            # 2048 gates
BT = BC * T          # 320 (t, b) rows per core

BROADCAST_ADD = True   # DVE tensor_tensor with free-dim step-0 broadcast


def build_program(has_bias: bool, has_outb: bool):
    import concourse.bacc as bacc
    import concourse.mybir as mybir
    import concourse.tile as tile

    dt = mybir.dt
    f32, f32r, b16 = dt.float32, dt.float32r, dt.bfloat16
    AF = mybir.ActivationFunctionType
    ALU = mybir.AluOpType
    AX = mybir.AxisListType

    nc = bacc.Bacc(
        "TRN2",
        target_bir_lowering=False,
        debug=False,
        enable_asserts=False,
        num_devices=NCORES,
    )

    def din(name, shape, dtype):
        return nc.dram_tensor(name, list(shape), dtype, kind="ExternalInput").ap()

    featsF_d = din("featsF", (128, KT, F), b16)       # flat bn tiles, padded
    featsT_d = din("featsT", (128, 4, NF), b16)       # [p, fk, bn] for fproj
    wft_d = din("wft", (128, 4, A), b16)
    wfb_d = din("wfb", (128, 4), f32)
    whb_d = din("whb", (128, 4), f32)
    whT_d = din("whT", (128, 4, A), b16)
    vt_d = din("vt", (128, 4), b16)
    eblk_d = din("eblk", (128, KT, BC), b16)          # indicator mask
    onesc_d = din("onesc", (128, 1), b16)
    wit_e_d = din("wit_e", (128, 4, G), b16)          # W_ih^T emb rows
    wit_c_d = din("wit_c", (128, 4, G), b16)         # W_ih^T ctx rows
    wht_d = din("wht", (128, 4, G), b16)
    emba_d = din("emba", (128, T, 4, BC), b16)
    h0t_d = din("h0t", (128, 4, BC), f32)
    c0_d = din("c0", (BC, H), f32)
    eyef_d = din("eyef", (BC, BC), f32)
    onesr_d = din("onesr", (1, 128), b16)
    biasrow_d = din("biasrow", (1, G), b16)
    owt_d = din("owt", (128, 4, V), b16)
    outb_d = din("outb", (1, V), b16)

    logits_d = nc.dram_tensor("logits", [BT, V], f32, kind="ExternalOutput").ap()

    with tile.TileContext(nc) as tc:
        with (
            tc.tile_pool(name="persist", bufs=1) as pp,
            tc.tile_pool(name="consts", bufs=1) as cp,
        ):
            hstate = pp.tile([128, 4, (T + 1) * BC], f32, tag="hstate")
            cst = pp.tile([BC, H], f32, tag="cst")
            srow = pp.tile([1, KT * 128], f32, tag="srow")

            eyef = cp.tile([BC, BC], f32, tag="eyef")
            vt = cp.tile([128, 4], b16, tag="vt")
            wfb = cp.tile([128, 4], f32, tag="wfb")
            whb = cp.tile([128, 4], f32, tag="whb")
            onesr = cp.tile([1, 128], b16, tag="onesr")
            onesc = cp.tile([128, 1], b16, tag="onesc")
            biasrow = cp.tile([1, G], b16, tag="biasrow")
            eblk = cp.tile([128, KT, BC], b16, tag="eblk")

            nc.sync.dma_start(eyef[:], eyef_d)
            nc.sync.dma_start(vt[:], vt_d)
            nc.sync.dma_start(wfb[:], wfb_d)
            nc.sync.dma_start(whb[:], whb_d)
            nc.sync.dma_start(onesr[:], onesr_d)
            nc.sync.dma_start(onesc[:], onesc_d)
            nc.sync.dma_start(biasrow[:], biasrow_d)
            nc.sync.dma_start(eblk[:], eblk_d)
            nc.sync.dma_start(hstate[:, :, 0:BC], h0t_d)
            nc.sync.dma_start(cst[:], c0_d)
            nc.vector.memset(srow[:], 0.0)

            with tc.tile_pool(name="phaseA", bufs=1) as pa:
                featsF = pa.tile([128, KT, F], b16, tag="featsF")
                fprojT = pa.tile([128, 4, BC, N], b16, tag="fprojT")
                wit_e = pa.tile([128, 4, G], b16, tag="wit_e")
                wit_c = pa.tile([128, 4, G], b16, tag="wit_c")
                wht_sb = pa.tile([128, 4, G], b16, tag="wht")
                whT_sb = pa.tile([128, 4, A], b16, tag="whT")
                emba = pa.tile([128, T, 4, BC], b16, tag="emba")

                nc.sync.dma_start(featsF[:], featsF_d)
                nc.sync.dma_start(wit_e[:], wit_e_d)
                nc.sync.dma_start(wit_c[:], wit_c_d)
                nc.sync.dma_start(wht_sb[:], wht_d)
                nc.sync.dma_start(whT_sb[:], whT_d)
                nc.sync.dma_start(emba[:], emba_d)

                # ---- setup: fprojT[a, bn] = Wf @ feats^T + Wf_b ----
                fprojT_fl = fprojT[:].rearrange("p a b n -> p a (b n)")
                with (
                    tc.tile_pool(name="setup", bufs=2) as sup,
                    tc.tile_pool(name="psF", bufs=2, space="PSUM") as psF,
                ):
                    wft_sb = sup.tile([128, 4, A], b16, tag="wft")
                    nc.sync.dma_start(wft_sb[:], wft_d)
                    CH = 512
                    nch = (NF + CH - 1) // CH
                    for j in range(nch):
                        c0c = j * CH
                        w = min(CH, NF - c0c)
                        ft = sup.tile([128, 4, CH], b16, tag="ft")
                        nc.sync.dma_start(
                            ft[:, :, 0:w], featsT_d[:, :, c0c : c0c + w]
                        )
                        for ac in range(4):
                            ps = psF.tile([128, CH], f32, tag="f")
                            for kk in range(4):
                                nc.tensor.matmul(
                                    ps[:, 0:w],
                                    wft_sb[:, kk, ac * 128 : (ac + 1) * 128],
                                    ft[:, kk, 0:w],
                                    start=(kk == 0),
                                    stop=(kk == 3),
                                )
                            nc.vector.tensor_scalar_add(
                                fprojT_fl[:, ac, c0c : c0c + w],
                                ps[:, 0:w],
                                wfb[:, ac : ac + 1],
                            )

                # ---- recurrent steps ----
                with (
                    tc.tile_pool(name="psM", bufs=1, space="PSUM") as psM,
                    tc.tile_pool(name="psS", bufs=2, space="PSUM") as psS,
                    tc.tile_pool(name="psC", bufs=1, space="PSUM") as psC,
                    tc.tile_pool(name="psT", bufs=1, space="PSUM") as psT,
                    tc.tile_pool(name="psG", bufs=2, space="PSUM") as psG,
                    tc.tile_pool(name="psB2", bufs=1, space="PSUM") as psB2,
                    tc.tile_pool(name="sm", bufs=2) as sm,
                    tc.tile_pool(name="smb", bufs=1) as smb,
                    tc.tile_pool(name="ep", bufs=2) as ep,
                    tc.tile_pool(name="et4", bufs=4) as et4,
                    tc.tile_pool(name="ilv", bufs=2) as ilv,
                    tc.tile_pool(name="ilh", bufs=1) as ilh,
                ):
                    # interleaved phase-B for logits rows 0..127 (t=0..7):
                    # spread over the PE gaps of steps 8..19
                    VCI = 512
                    nvci = (V + VCI - 1) // VCI
                    ilv_plan = {}
                    done = 0
                    for t in range(8, T):
                        n = min(nvci - done, (nvci + (T - 8) - 1) // (T - 8))
                        ilv_plan[t] = range(done, done + n)
                        done += n
                    hm0 = None
                    for t in range(T):
                        hcol = hstate[:, :, t * BC : (t + 1) * BC]

                        # h in bf16 for the attention projection
                        h_b16 = sm.tile([128, 4, BC], b16, tag="h_b16")
                        nc.vector.tensor_copy(h_b16[:], hcol)
                        # hproj[a, b] = Wh @ h + Wh_b
                        hp = psM.tile([128, 4 * BC], f32, tag="m")
                        for ac in range(4):
                            for hk in range(4):
                                nc.tensor.matmul(
                                    hp[:, ac * BC : (ac + 1) * BC],
                                    whT_sb[:, hk, ac * 128 : (ac + 1) * 128],
                                    h_b16[:, hk, :],
                                    start=(hk == 0),
                                    stop=(hk == 3),
                                )
                        hp_sb = sm.tile([128, 4, BC], f32, tag="hp_sb")
                        for ac in range(4):
                            nc.vector.tensor_scalar_add(
                                hp_sb[:, ac, :],
                                hp[:, ac * BC : (ac + 1) * BC],
                                whb[:, ac : ac + 1],
                            )

                        # e = tanh(fproj + hproj)
                        e_ts = []
                        for ac in range(4):
                            e_in = ep.tile([128, BC, N], b16, tag="e_in")
                            if BROADCAST_ADD:
                                for hlf in range(2):
                                    hsl = slice(hlf * (BC // 2), (hlf + 1) * (BC // 2))
                                    h_bc = (
                                        hp_sb[:, ac, hsl]
                                        .unsqueeze(2)
                                        .broadcast_to((128, BC // 2, N))
                                    )
                                    nc.vector.tensor_add(
                                        e_in[:, hsl, :], fprojT[:, ac, hsl, :], h_bc
                                    )
                            else:
                                for b in range(BC):
                                    nc.vector.tensor_scalar_add(
                                        e_in[:, b, :],
                                        fprojT[:, ac, b, :],
                                        hp_sb[:, ac, b : b + 1],
                                    )
                            e_t = et4.tile([128, BC, N], b16, tag="e_t")
                            for hlf in range(2):
                                hsl = slice(hlf * (BC // 2), (hlf + 1) * (BC // 2))
                                nc.scalar.activation(
                                    e_t[:, hsl, :], e_in[:, hsl, :], AF.Tanh
                                )
                            e_ts.append(e_t)

                        # scores (flat bn) -> srow (staging row)
                        CH = 512
                        nch = (NF + CH - 1) // CH
                        for j in range(nch):
                            c0c = j * CH
                            w = min(CH, NF - c0c)
                            ps = psS.tile([1, CH], f32, tag="s")
                            for ac in range(4):
                                efl = e_ts[ac][:].rearrange(
                                    "p b n -> p (b n)"
                                )
                                nc.tensor.matmul(
                                    ps[0:1, 0:w],
                                    vt[:, ac : ac + 1],
                                    efl[:, c0c : c0c + w],
                                    start=(ac == 0),
                                    stop=(ac == 3),
                                )
                            nc.vector.tensor_copy(
                                srow[0:1, c0c : c0c + w], ps[0:1, 0:w]
                            )
                        # reshape srow -> [128, KT] via PE transposes
                        pst = psM.tile([128, KT], f32, tag="m")
                        for k in range(KT):
                            nc.tensor.transpose(
                                pst[:, k : k + 1],
                                srow[0:1, k * 128 : (k + 1) * 128],
                                eyef[0:1, 0:1],
                            )
                        # softmax pieces: exp (no max shift; scores are small)
                        expf = sm.tile([128, KT], f32, tag="expf")
                        nc.scalar.activation(expf[:], pst[:], AF.Exp)
                        # wblk[p, k, b] = eblk * expf (masked weights)
                        wblk = sm.tile([128, KT, BC], b16, tag="wblk")
                        if BROADCAST_ADD:
                            ex_bc = (
                                expf[:].unsqueeze(2).broadcast_to((128, KT, BC))
                            )
                            nc.vector.tensor_mul(wblk[:], eblk[:], ex_bc)
                        else:
                            for k in range(KT):
                                nc.vector.tensor_scalar_mul(
                                    wblk[:, k, :],
                                    eblk[:, k, :],
                                    expf[:, k : k + 1],
                                )

                        # normalizer: one matmul -> per-(k,b) column sums,
                        # segmented reduce over k, transpose back to [b, 1]
                        psum_sum = psM.tile([1, KT * BC], f32, tag="m")
                        nc.tensor.matmul(
                            psum_sum[:],
                            onesc[:],
                            wblk[:].rearrange("p k b -> p (k b)"),
                            start=True,
                            stop=True,
                        )
                        sumrow = sm.tile([1, BC], f32, tag="sumrow")
                        nc.vector.tensor_reduce(
                            sumrow[:].unsqueeze(2),
                            psum_sum[:].rearrange("p (k b) -> p b k", k=KT),
                            axis=AX.X,
                            op=ALU.add,
                        )
                        psr = psM.tile([BC, 1], f32, tag="m")
                        nc.tensor.transpose(
                            psr[:], sumrow[:], eyef[0:1, 0:1]
                        )
                        rinv = sm.tile([BC, 1], f32, tag="rinv")
                        nc.vector.reciprocal(rinv[:], psr[:])
                        # unnormalized ctx
                        ps_cu = psC.tile([BC, F], f32, tag="c")
                        for k in range(KT):
                            nc.tensor.matmul(
                                ps_cu[:],
                                wblk[:, k, :],
                                featsF[:, k, :],
                                start=(k == 0),
                                stop=(k == KT - 1),
                            )
                        ctx_sb = smb.tile([BC, F], f32, tag="ctx_sb")
                        nc.vector.tensor_scalar_mul(
                            ctx_sb[:], ps_cu[:], rinv[:]
                        )
                        xctx = sm.tile([128, 4, BC], b16, tag="xctx")
                        for kk in range(4):
                            pc = psT.tile([128, BC], f32, tag="tr")
                            nc.tensor.transpose(
                                pc[:], ctx_sb[:, kk * 128 : (kk + 1) * 128],
                                eyef[:],
                            )
                            nc.vector.tensor_copy(xctx[:, kk, :], pc[:])

                        # gates (i, f, g, o) + pointwise
                        tig = smb.tile([BC, 4, H], f32, tag="tig")
                        for g in range(4):
                            pg = psG.tile([BC, H], f32, tag="g")
                            gsl = slice(g * H, (g + 1) * H)
                            for kk in range(4):
                                nc.tensor.matmul(
                                    pg[:], emba[:, t, kk, :], wit_e[:, kk, gsl],
                                    start=(kk == 0), stop=False,
                                )
                            for kk in range(4):
                                nc.tensor.matmul(
                                    pg[:], xctx[:, kk, :], wit_c[:, kk, gsl],
                                    start=False, stop=False,
                                )
                            for hk in range(4):
                                last = (hk == 3) and not has_bias
                                nc.tensor.matmul(
                                    pg[:], h_b16[:, hk, :], wht_sb[:, hk, gsl],
                                    start=False, stop=last,
                                )
                            if has_bias:
                                nc.tensor.matmul(
                                    pg[:], onesr[0:1, 0:BC], biasrow[0:1, gsl],
                                    start=False, stop=True,
                                )
                            scale = 1.0 if g == 2 else 0.5
                            nc.scalar.activation(
                                tig[:, g, :], pg[:], AF.Tanh, scale=scale
                            )

                        # c = sig(f)*c + sig(i)*tanh(g); h = sig(o)*tanh(c)
                        for g in (0, 1, 3):
                            nc.vector.tensor_scalar(
                                tig[:, g, :], tig[:, g, :], 1.0, 0.5,
                                op0=ALU.add, op1=ALU.mult,
                            )
                        tmp1 = smb.tile([BC, H], f32, tag="u1")
                        nc.vector.tensor_mul(tmp1[:], tig[:, 1, :], cst[:])
                        tmp2 = smb.tile([BC, H], f32, tag="u2")
                        nc.vector.tensor_mul(tmp2[:], tig[:, 0, :], tig[:, 2, :])
                        nc.vector.tensor_add(cst[:], tmp1[:], tmp2[:])
                        tcn = smb.tile([BC, H], f32, tag="u1")
                        nc.scalar.activation(tcn[:], cst[:], AF.Tanh)
                        hn = smb.tile([BC, H], f32, tag="u2")
                        nc.vector.tensor_mul(hn[:], tig[:, 3, :], tcn[:])

                        for kk in range(4):
                            ph = psT.tile([128, BC], f32, tag="tr")
                            nc.tensor.transpose(
                                ph[:], hn[:, kk * 128 : (kk + 1) * 128], eyef[:]
                            )
                            nc.vector.tensor_copy(
                                hstate[:, kk, (t + 1) * BC : (t + 2) * BC],
                                ph[:],
                            )
                        if t == 7:
                            hm0 = ilh.tile([128, 4, 128], b16, tag="hm0")
                            nc.vector.tensor_copy(
                                hm0[:], hstate[:, :, BC : BC + 128]
                            )
                        for ic, vc in enumerate(ilv_plan.get(t, ())):
                            v0 = vc * VCI
                            vw = min(VCI, V - v0)
                            ow2 = ilv.tile([128, 4, VCI], b16, tag="ow2")
                            nc.sync.dma_start(
                                ow2[:, :, 0:vw], owt_d[:, :, v0 : v0 + vw]
                            )
                            pb = psB2.tile([128, VCI], f32, tag="b2")
                            for hk in range(4):
                                nc.tensor.matmul(
                                    pb[:, 0:vw], hm0[:, hk, :],
                                    ow2[:, hk, 0:vw],
                                    start=(hk == 0), stop=(hk == 3),
                                )
                            lb2 = ilv.tile([128, VCI], f32, tag="lb2")
                            if ic % 2 == 0:
                                nc.vector.tensor_copy(lb2[:, 0:vw], pb[:, 0:vw])
                            else:
                                nc.scalar.copy(lb2[:, 0:vw], pb[:, 0:vw])
                            nc.sync.dma_start(
                                logits_d[0:128, v0 : v0 + vw], lb2[:, 0:vw]
                            )

            # ---- phase B: logits = h @ out_w^T (+ out_b) ----
            with (
                tc.tile_pool(name="owp", bufs=3) as owp,
                tc.tile_pool(name="psB", bufs=4, space="PSUM") as psB,
                tc.tile_pool(name="obp", bufs=1) as obp,
                tc.tile_pool(name="lbp", bufs=4) as lbp,
            ):
                hb16 = obp.tile([128, 4, T * BC], b16, tag="hb16")
                nc.vector.tensor_copy(hb16[:], hstate[:, :, BC:])
                if has_outb:
                    outb_sb = obp.tile([1, V], b16, tag="outb")
                    nc.sync.dma_start(outb_sb[:], outb_d)
                VC = 2048
                nvc = (V + VC - 1) // VC
                mtiles = [(128, 128), (256, 64)]
                for vc in range(nvc):
                    v0 = vc * VC
                    vw = min(VC, V - v0)
                    ow = owp.tile([128, 4, VC], b16, tag="ow")
                    nc.sync.dma_start(ow[:, :, 0:vw], owt_d[:, :, v0 : v0 + vw])
                    for m0, mr in mtiles:
                        lb = lbp.tile([128, VC], f32, tag="lb")
                        for ih, half in enumerate(range(0, vw, 512)):
                            hw_ = min(512, vw - half)
                            ps = psB.tile([128, 512], f32, tag="pb")
                            for hk in range(4):
                                last = (hk == 3) and not has_outb
                                nc.tensor.matmul(
                                    ps[0:mr, 0:hw_],
                                    hb16[:, hk, m0 : m0 + mr],
                                    ow[:, hk, half : half + hw_],
                                    start=(hk == 0), stop=last,
                                )
                            if has_outb:
                                nc.tensor.matmul(
                                    ps[0:mr, 0:hw_],
                                    onesr[0:1, 0:mr],
                                    outb_sb[0:1, v0 + half : v0 + half + hw_],
                                    start=False, stop=True,
                                )
                            ceng = nc.vector if ih % 2 == 0 else nc.scalar
                            if ceng is nc.vector:
                                ceng.tensor_copy(
                                    lb[0:mr, half : half + hw_], ps[0:mr, 0:hw_]
                                )
                            else:
                                nc.scalar.copy(
                                    lb[0:mr, half : half + hw_], ps[0:mr, 0:hw_]
                                )
                        nc.sync.dma_start(
                            logits_d[m0 : m0 + mr, v0 : v0 + vw],
                            lb[0:mr, 0:vw],
                        )

    nc.compile()
    return nc


def _tileize(mat, ntiles):
    """[K, M] -> [128, ntiles, M]: row k = kk*128+p -> [p, kk, :]."""
    K, M = mat.shape
    assert K == ntiles * 128
    return np.ascontiguousarray(mat.reshape(ntiles, 128, M).transpose(1, 0, 2))


def prep_inputs(feats, captions, embed_W, Wf_w, Wf_b, Wh_w, Wh_b, v_w, v_b,
                W_ih, W_hh, b_ih, b_hh, init_w, init_b, initc_w, initc_b,
                out_w, out_b):
    feats = np.asarray(feats, np.float32)
    captions = np.asarray(captions)
    emtab = np.array(embed_W, np.float32)
    emtab[0] = 0.0
    emb = emtab[captions[:, :-1]]                      # [B, T, E]
    gfeat = feats.mean(axis=1, dtype=np.float32)
    h0 = (gfeat @ np.asarray(init_w, np.float32).T + init_b).astype(np.float32)
    c0 = (gfeat @ np.asarray(initc_w, np.float32).T + initc_b).astype(np.float32)

    witT = np.asarray(W_ih, np.float32).T              # [E+F, G]
    wit_e = _tileize(witT[0:E], 4).astype(bf)
    wit_c = _tileize(witT[E:], 4).astype(bf)
    wht = _tileize(np.asarray(W_hh, np.float32).T, 4).astype(bf)
    wft = _tileize(np.asarray(Wf_w, np.float32).T, 4).astype(bf)
    whT = _tileize(np.asarray(Wh_w, np.float32).T, 4).astype(bf)
    owt = _tileize(np.asarray(out_w, np.float32).T, 4).astype(bf)
    wfb = np.ascontiguousarray(np.asarray(Wf_b, np.float32).reshape(4, 128).T)
    whb = np.ascontiguousarray(np.asarray(Wh_b, np.float32).reshape(4, 128).T)
    vt = np.ascontiguousarray(
        np.asarray(v_w, np.float32)[0].reshape(4, 128).T
    ).astype(bf)
    biasrow = (np.asarray(b_ih, np.float32) + np.asarray(b_hh, np.float32))[
        None, :
    ].astype(bf)
    outb = np.asarray(out_b, np.float32)[None, :].astype(bf)
    onesr = np.ones((1, 128), bf)
    onesc = np.ones((128, 1), bf)
    eyef = np.eye(BC, dtype=np.float32)

    # indicator: flat row 128k+p belongs to sample b
    idx = np.arange(KT * 128)
    bown = idx // N
    eblk = np.zeros((KT * 128, BC), np.float32)
    valid = idx < NF
    eblk[valid, bown[valid]] = 1.0
    eblk = np.ascontiguousarray(
        eblk.reshape(KT, 128, BC).transpose(1, 0, 2)
    ).astype(bf)

    # NOTE: v_b shifts all scores of a row equally -> softmax-invariant.

    in_maps = []
    for c in range(NCORES):
        s = slice(c * BC, (c + 1) * BC)
        fc = feats[s]                                   # [16, 196, 512]
        ff = fc.reshape(NF, F)
        featsF = np.zeros((KT * 128, F), np.float32)
        featsF[0:NF] = ff
        featsF = np.ascontiguousarray(
            featsF.reshape(KT, 128, F).transpose(1, 0, 2)
        ).astype(bf)
        featsT = np.ascontiguousarray(
            ff.T.reshape(4, 128, NF).transpose(1, 0, 2)
        ).astype(bf)
        emba = np.ascontiguousarray(
            emb[s].reshape(BC, T, 4, 128).transpose(3, 1, 2, 0)
        ).astype(bf)
        h0t = np.ascontiguousarray(
            h0[s].reshape(BC, 4, 128).transpose(2, 1, 0)
        )
        in_maps.append({
            "featsF": featsF, "featsT": featsT, "wft": wft, "wfb": wfb,
            "whb": whb, "whT": whT, "vt": vt, "eblk": eblk, "onesc": onesc,
            "wit_e": wit_e, "wit_c": wit_c, "wht": wht, "emba": emba,
            "h0t": h0t, "c0": np.ascontiguousarray(c0[s]), "eyef": eyef,
            "onesr": onesr, "biasrow": biasrow, "owt": owt, "outb": outb,
        })
    return in_maps, bool(np.any(biasrow)), bool(np.any(outb))


def assemble(results):
    out = np.empty((B, T, V), np.float32)
    for c, r in enumerate(results):
        out[c * BC : (c + 1) * BC] = (
            r["logits"].reshape(T, BC, V).transpose(1, 0, 2)
        )
    return out


_CACHE = {}
LAST_RESULTS = None


def kernel(**inputs):
    global LAST_RESULTS
    from concourse.bass_utils import run_bass_kernel_spmd

    in_maps, has_bias, has_outb = prep_inputs(**inputs)
    key = (has_bias, has_outb)
    if key not in _CACHE:
        _CACHE[key] = build_program(has_bias, has_outb)
    res = run_bass_kernel_spmd(
        _CACHE[key], in_maps, list(range(NCORES)),
        trace=bool(int(os.environ.get("KTRACE", "0"))),
    )
    LAST_RESULTS = res
    return assemble(res.results)
